# revision 29
# baseline (speedup 1.0000x reference)
"""Trainium2 Bass kernel for the Flux_Kernels 5-point Dirichlet stencil.

out[i,j] = D*s0*(u[i-1,j] + u[i+1,j] + u[i,j-1] + u[i,j+1]) + 4*D*s1*u[i,j]
with out-of-range neighbors replaced by dirichlet_val[{0,1,2,3}].

Strategy: pad u with the Dirichlet constants into S [4098, 4098] on the host,
shard along rows: core k gets S[512k : 512k+514] (1-row halo each side baked
into the slab). On each core, tiles of 128 consecutive padded rows are
processed with partition p <-> padded row r0+p:
  - TensorE: tridiagonal matmul W.T @ tile -> PSUM[p] = a*up + c*ctr + a*down
    centered at padded row r0+p (rows 0 and 127 are incomplete and discarded)
  - VectorE: lr[p] = tile[p, j] + tile[p, j+2]  (left+right sums)
  - VectorE: o[p] = (lr[p] * a) + PSUM[p]       (fused scalar_tensor_tensor,
    also evacuates PSUM)
  - output DMA stores partitions 1..126 -> 126 output rows per tile; the
    DMA absorbs the one-row shift that compute engines cannot express.
    Stores are issued on the ACT HWDGE ring so they never head-of-line
    block input prefetches on the SP ring.
Consecutive tiles overlap by 2 rows. The 8-row remainder tile is reshaped
into 4 column-blocks of 1024 placed at partition bases {0,32,64,96} so its
vector work engages 128 partitions instead of 10. All scalars (a = D*s0,
c = 4*D*s1, weight matrices) are computed on the host from runtime inputs;
the per-partition coefficient `a` rides as an extra column of w_main.
"""

import sys

import numpy as np

if "/opt/trn_rl_repo" not in sys.path:
    sys.path.insert(0, "/opt/trn_rl_repo")

NX, NY = 4096, 4096
N_CORES = 8
ROWS_PER_CORE = NX // N_CORES          # 512
SLAB_ROWS = ROWS_PER_CORE + 2          # 514
PAD_COLS = NY + 2                      # 4098
TILE_OUT = 126                         # output rows per full tile
FULL_TILES = ROWS_PER_CORE // TILE_OUT  # 4
LAST_OUT = ROWS_PER_CORE - FULL_TILES * TILE_OUT  # 8
LAST_IN = LAST_OUT + 2                 # 10
LAST_R0 = FULL_TILES * TILE_OUT        # 504
PSUM_CHUNK = 2048                      # free-dim columns per PSUM tile
MM_N = 512                             # matmul moving free dim (1 PSUM bank)
BLK = 2048                             # tile-4 column-block width

_CACHE: dict = {}


def _build_nc():
    import concourse.bass as bass
    import concourse.mybir as mybir
    from concourse import bacc
    from concourse.tile import TileContext

    f32 = mybir.dt.float32
    add = mybir.AluOpType.add
    mult = mybir.AluOpType.mult

    nc = bacc.Bacc(None, target_bir_lowering=False)
    s_in = nc.dram_tensor("s_in", (SLAB_ROWS, PAD_COLS), f32, kind="ExternalInput")
    w_main = nc.dram_tensor("w_main", (128, 129), f32, kind="ExternalInput")
    w_aux = nc.dram_tensor("w_aux", (128, LAST_IN), f32, kind="ExternalInput")
    out = nc.dram_tensor("out", (ROWS_PER_CORE, NY), f32, kind="ExternalOutput")

    with TileContext(nc) as tc:
        with (
            tc.tile_pool(name="const", bufs=1) as cpool,
            tc.tile_pool(name="inp", bufs=4) as ipool,
            tc.tile_pool(name="lrp", bufs=1) as lpool,
            tc.tile_pool(name="lrg", bufs=2) as gpool,
            tc.tile_pool(name="op", bufs=3) as opool,
            tc.tile_pool(name="psum", bufs=2, space=bass.MemorySpace.PSUM) as ppool,
        ):
            in_tiles = [
                ipool.tile([128, PAD_COLS], f32, tag="in", name=f"in{i}")
                for i in range(4)
            ]
            # tile-0 input first so DVE can start the moment it boots
            nc.sync.dma_start(out=in_tiles[0][:], in_=s_in[0:128, :])

            w_t = cpool.tile([128, 129], f32)
            nc.sync.dma_start(out=w_t[:], in_=w_main[:])
            w5_t = cpool.tile([128, LAST_IN], f32)
            nc.sync.dma_start(out=w5_t[:], in_=w_aux[:])
            coef = w_t[:, 128:129]

            nc.sync.dma_start(out=in_tiles[1][:], in_=s_in[TILE_OUT : TILE_OUT + 128, :])
            nc.sync.dma_start(
                out=in_tiles[2][:], in_=s_in[2 * TILE_OUT : 2 * TILE_OUT + 128, :]
            )
            # remainder tile early: 2 column-blocks at partition bases {0, 64}
            in5 = ipool.tile([128, BLK + 2], f32, tag="in5", bufs=1)
            for cb in range(2):
                nc.sync.dma_start(
                    out=in5[64 * cb : 64 * cb + LAST_IN, :],
                    in_=s_in[LAST_R0:SLAB_ROWS, BLK * cb : BLK * cb + BLK + 2],
                )
            nc.sync.dma_start(
                out=in_tiles[3][:], in_=s_in[3 * TILE_OUT : 3 * TILE_OUT + 128, :]
            )

            def full_tile(t):
                r0 = TILE_OUT * t
                in_t = in_tiles[t]
                # left+right sums. Tile 0 on DVE (earliest start); tiles 1-3
                # on the otherwise-idle GpSimd, whose 2-src adds contend with
                # DVE 2-src adds but not with the STT combines — so after
                # tile 0 the DVE runs combines only.
                if t < 1:
                    lr_t = lpool.tile([128, NY], f32, tag="lr", name=f"lr{t}")
                    nc.vector.tensor_add(
                        out=lr_t[:], in0=in_t[:, 0:NY], in1=in_t[:, 2 : NY + 2]
                    )
                else:
                    lr_t = gpool.tile([128, NY], f32, tag="lrg", name=f"lrg{t}")
                    nc.gpsimd.tensor_add(
                        out=lr_t[:], in0=in_t[:, 0:NY], in1=in_t[:, 2 : NY + 2]
                    )

                o_t = opool.tile([128, NY], f32, tag="o", name=f"o{t}")
                for h in range(NY // PSUM_CHUNK):
                    ps = ppool.tile([128, PSUM_CHUNK], f32, tag="ps", name=f"ps{t}{h}")
                    for q in range(PSUM_CHUNK // MM_N):
                        cc = h * PSUM_CHUNK + q * MM_N
                        # a*up + c*ctr + a*down (rows via tridiagonal weights)
                        nc.tensor.matmul(
                            ps[:, q * MM_N : (q + 1) * MM_N],
                            w_t[:, 0:128],
                            in_t[:, 1 + cc : 1 + cc + MM_N],
                            start=True,
                            stop=True,
                        )
                    # o = a*(left+right) + psum; also evacuates PSUM
                    nc.vector.scalar_tensor_tensor(
                        out=o_t[:, h * PSUM_CHUNK : (h + 1) * PSUM_CHUNK],
                        in0=lr_t[:, h * PSUM_CHUNK : (h + 1) * PSUM_CHUNK],
                        scalar=coef,
                        in1=ps[:, :],
                        op0=mult,
                        op1=add,
                    )
                    # stores ride the ACT HWDGE ring (SP ring is for loads)
                    nc.scalar.dma_start(
                        out=out[r0 : r0 + TILE_OUT, h * PSUM_CHUNK : (h + 1) * PSUM_CHUNK],
                        in_=o_t[1 : 1 + TILE_OUT, h * PSUM_CHUNK : (h + 1) * PSUM_CHUNK],
                    )

            def last_tile():
                # partition 64*cb + r <-> padded row 504+r,
                # columns [2048*cb, 2048*cb + 2049]
                lr5 = gpool.tile([128, BLK], f32, tag="lrg5", bufs=1)
                nc.gpsimd.tensor_add(
                    out=lr5[:], in0=in5[:, 0:BLK], in1=in5[:, 2 : BLK + 2]
                )
                o5 = opool.tile([128, BLK], f32, tag="o")
                ps5 = ppool.tile([128, BLK], f32, tag="ps")
                for q in range(BLK // MM_N):
                    for cb in range(2):
                        nc.tensor.matmul(
                            ps5[64 * cb : 64 * cb + LAST_IN, q * MM_N : (q + 1) * MM_N],
                            w5_t[64 * cb : 64 * cb + LAST_IN, :],
                            in5[64 * cb : 64 * cb + LAST_IN, 1 + q * MM_N : 1 + (q + 1) * MM_N],
                            start=True,
                            stop=True,
                        )
                nc.vector.scalar_tensor_tensor(
                    out=o5[:, :],
                    in0=lr5[:, :],
                    scalar=coef,
                    in1=ps5[:, :],
                    op0=mult,
                    op1=add,
                )
                for cb in range(2):
                    nc.scalar.dma_start(
                        out=out[LAST_R0 : LAST_R0 + LAST_OUT, BLK * cb : BLK * (cb + 1)],
                        in_=o5[64 * cb + 1 : 64 * cb + 1 + LAST_OUT, :],
                    )

            full_tile(0)
            full_tile(1)
            full_tile(2)
            full_tile(3)
            last_tile()

    nc.compile()
    return nc


def _get_nc():
    if "nc" not in _CACHE:
        _CACHE["nc"] = _build_nc()
    return _CACHE["nc"]


def _tridiag(n, a, c):
    w = np.zeros((n, n), dtype=np.float32)
    i = np.arange(n)
    w[i, i] = c
    w[i[:-1], i[1:]] = a  # k = m-1 (up neighbor)
    w[i[1:], i[:-1]] = a  # k = m+1 (down neighbor)
    return w


def _weight_inputs(a, c):
    w_main = np.empty((128, 129), dtype=np.float32)
    w_main[:, 0:128] = _tridiag(128, a, c)
    w_main[:, 128] = a  # per-partition STT coefficient
    w_aux = np.zeros((128, LAST_IN), dtype=np.float32)
    for cb in range(2):
        w_aux[64 * cb : 64 * cb + LAST_IN, :] = _tridiag(LAST_IN, a, c)
    return {"w_main": w_main, "w_aux": w_aux}


def kernel(u_main, u_coupled=None, D_eff=None, dirichlet_val=None, stencil=None,
           t=None, **_ignored):
    u = np.asarray(u_main, dtype=np.float32)
    assert u.shape == (NX, NY), u.shape
    D = float(np.asarray(D_eff).reshape(-1)[0])
    st = np.asarray(stencil).reshape(-1)
    s0, s1 = float(st[0]), float(st[1])
    dv = np.asarray(dirichlet_val, dtype=np.float32).reshape(-1)
    a = np.float32(D * s0)
    c = np.float32(4.0 * D * s1)

    S = np.empty((NX + 2, NY + 2), dtype=np.float32)
    S[1:-1, 1:-1] = u
    S[0, :] = dv[0]       # x- boundary (row 0 up-neighbor)
    S[-1, :] = dv[1]      # x+ boundary
    S[1:-1, 0] = dv[2]    # y- boundary
    S[1:-1, -1] = dv[3]   # y+ boundary

    in_maps = [
        {
            "s_in": np.ascontiguousarray(S[ROWS_PER_CORE * k : ROWS_PER_CORE * k + SLAB_ROWS]),
            **_weight_inputs(a, c),
        }
        for k in range(N_CORES)
    ]

    from concourse.bass_utils import run_bass_kernel_spmd

    res = run_bass_kernel_spmd(_get_nc(), in_maps, core_ids=list(range(N_CORES)))
    return np.concatenate([r["out"] for r in res.results], axis=0)


# revision 31
# speedup vs baseline: 1.1175x; 1.1175x over previous
"""Trainium2 Bass kernel for the Flux_Kernels 5-point Dirichlet stencil.

out[i,j] = D*s0*(u[i-1,j] + u[i+1,j] + u[i,j-1] + u[i,j+1]) + 4*D*s1*u[i,j]
with out-of-range neighbors replaced by dirichlet_val[{0,1,2,3}].

Strategy: pad u with the Dirichlet constants into S [4098, 4098] on the host,
shard along rows: core k gets S[512k : 512k+514] (1-row halo each side baked
into the slab). On each core, tiles of 128 consecutive padded rows are
processed with partition p <-> padded row r0+p:
  - TensorE: tridiagonal matmul W.T @ tile -> PSUM[p] = a*up + c*ctr + a*down
    centered at padded row r0+p (rows 0 and 127 are incomplete and discarded)
  - VectorE: lr[p] = tile[p, j] + tile[p, j+2]  (left+right sums)
  - VectorE: o[p] = (lr[p] * a) + PSUM[p]       (fused scalar_tensor_tensor,
    also evacuates PSUM)
  - output DMA stores partitions 1..126 -> 126 output rows per tile; the
    DMA absorbs the one-row shift that compute engines cannot express.
    Stores are issued on the ACT HWDGE ring so they never head-of-line
    block input prefetches on the SP ring.
Consecutive tiles overlap by 2 rows. The 8-row remainder tile is reshaped
into 4 column-blocks of 1024 placed at partition bases {0,32,64,96} so its
vector work engages 128 partitions instead of 10. All scalars (a = D*s0,
c = 4*D*s1, weight matrices) are computed on the host from runtime inputs;
the per-partition coefficient `a` rides as an extra column of w_main.
"""

import sys

import numpy as np

if "/opt/trn_rl_repo" not in sys.path:
    sys.path.insert(0, "/opt/trn_rl_repo")

NX, NY = 4096, 4096
N_CORES = 8
ROWS_PER_CORE = NX // N_CORES          # 512
SLAB_ROWS = ROWS_PER_CORE + 2          # 514
PAD_COLS = NY + 2                      # 4098
TILE_OUT = 126                         # output rows per full tile
FULL_TILES = ROWS_PER_CORE // TILE_OUT  # 4
LAST_OUT = ROWS_PER_CORE - FULL_TILES * TILE_OUT  # 8
LAST_IN = LAST_OUT + 2                 # 10
LAST_R0 = FULL_TILES * TILE_OUT        # 504
PSUM_CHUNK = 2048                      # free-dim columns per PSUM tile
MM_N = 512                             # matmul moving free dim (1 PSUM bank)
BLK = 2048                             # tile-4 column-block width

_CACHE: dict = {}


def _build_nc():
    import concourse.bass as bass
    import concourse.mybir as mybir
    from concourse import bacc
    from concourse.tile import TileContext

    f32 = mybir.dt.float32
    add = mybir.AluOpType.add
    mult = mybir.AluOpType.mult

    nc = bacc.Bacc(None, target_bir_lowering=False)
    s_in = nc.dram_tensor("s_in", (SLAB_ROWS, PAD_COLS), f32, kind="ExternalInput")
    w_main = nc.dram_tensor("w_main", (128, 129), f32, kind="ExternalInput")
    w_aux = nc.dram_tensor("w_aux", (128, LAST_IN), f32, kind="ExternalInput")
    out = nc.dram_tensor("out", (ROWS_PER_CORE, NY), f32, kind="ExternalOutput")

    with TileContext(nc) as tc:
        with (
            tc.tile_pool(name="const", bufs=1) as cpool,
            tc.tile_pool(name="inp", bufs=4) as ipool,
            tc.tile_pool(name="lrp", bufs=1) as lpool,
            tc.tile_pool(name="lrg", bufs=2) as gpool,
            tc.tile_pool(name="op", bufs=3) as opool,
            tc.tile_pool(name="psum", bufs=2, space=bass.MemorySpace.PSUM) as ppool,
        ):
            in_tiles = [
                ipool.tile([128, PAD_COLS], f32, tag="in", name=f"in{i}")
                for i in range(4)
            ]
            # tile-0 input first so DVE can start the moment it boots
            nc.sync.dma_start(out=in_tiles[0][:], in_=s_in[0:128, :])

            w_t = cpool.tile([128, 129], f32)
            nc.sync.dma_start(out=w_t[:], in_=w_main[:])
            w5_t = cpool.tile([128, LAST_IN], f32)
            nc.sync.dma_start(out=w5_t[:], in_=w_aux[:])
            coef = w_t[:, 128:129]

            nc.sync.dma_start(out=in_tiles[1][:], in_=s_in[TILE_OUT : TILE_OUT + 128, :])
            nc.sync.dma_start(
                out=in_tiles[2][:], in_=s_in[2 * TILE_OUT : 2 * TILE_OUT + 128, :]
            )
            nc.sync.dma_start(
                out=in_tiles[3][:], in_=s_in[3 * TILE_OUT : 3 * TILE_OUT + 128, :]
            )
            # remainder tile: 2 column-blocks at partition bases {0, 64}
            in5 = ipool.tile([128, BLK + 2], f32, tag="in5", bufs=1)
            for cb in range(2):
                nc.sync.dma_start(
                    out=in5[64 * cb : 64 * cb + LAST_IN, :],
                    in_=s_in[LAST_R0:SLAB_ROWS, BLK * cb : BLK * cb + BLK + 2],
                )

            def full_tile(t):
                r0 = TILE_OUT * t
                in_t = in_tiles[t]
                # left+right sums. Tile 0 on DVE (earliest start); tiles 1-3
                # on the otherwise-idle GpSimd, whose 2-src adds contend with
                # DVE 2-src adds but not with the STT combines — so after
                # tile 0 the DVE runs combines only.
                if t < 1:
                    lr_t = lpool.tile([128, NY], f32, tag="lr", name=f"lr{t}")
                    nc.vector.tensor_add(
                        out=lr_t[:], in0=in_t[:, 0:NY], in1=in_t[:, 2 : NY + 2]
                    )
                else:
                    lr_t = gpool.tile([128, NY], f32, tag="lrg", name=f"lrg{t}")
                    nc.gpsimd.tensor_add(
                        out=lr_t[:], in0=in_t[:, 0:NY], in1=in_t[:, 2 : NY + 2]
                    )

                o_t = opool.tile([128, NY], f32, tag="o", name=f"o{t}")
                for h in range(NY // PSUM_CHUNK):
                    ps = ppool.tile([128, PSUM_CHUNK], f32, tag="ps", name=f"ps{t}{h}")
                    for q in range(PSUM_CHUNK // MM_N):
                        cc = h * PSUM_CHUNK + q * MM_N
                        # a*up + c*ctr + a*down (rows via tridiagonal weights)
                        nc.tensor.matmul(
                            ps[:, q * MM_N : (q + 1) * MM_N],
                            w_t[:, 0:128],
                            in_t[:, 1 + cc : 1 + cc + MM_N],
                            start=True,
                            stop=True,
                        )
                    # o = a*(left+right) + psum; also evacuates PSUM
                    nc.vector.scalar_tensor_tensor(
                        out=o_t[:, h * PSUM_CHUNK : (h + 1) * PSUM_CHUNK],
                        in0=lr_t[:, h * PSUM_CHUNK : (h + 1) * PSUM_CHUNK],
                        scalar=coef,
                        in1=ps[:, :],
                        op0=mult,
                        op1=add,
                    )
                    # stores ride the ACT HWDGE ring (SP ring is for loads)
                    nc.scalar.dma_start(
                        out=out[r0 : r0 + TILE_OUT, h * PSUM_CHUNK : (h + 1) * PSUM_CHUNK],
                        in_=o_t[1 : 1 + TILE_OUT, h * PSUM_CHUNK : (h + 1) * PSUM_CHUNK],
                    )

            def last_tile():
                # partition 64*cb + r <-> padded row 504+r,
                # columns [2048*cb, 2048*cb + 2049]
                # sharing the lrg tag makes ADD5 wait for an lrg slot, which
                # keeps GpSimd's stream ordered ADD1, ADD2, ADD3, ADD5
                lr5 = gpool.tile([128, BLK], f32, tag="lrg")
                nc.gpsimd.tensor_add(
                    out=lr5[:], in0=in5[:, 0:BLK], in1=in5[:, 2 : BLK + 2]
                )
                o5 = opool.tile([128, BLK], f32, tag="o")
                ps5 = ppool.tile([128, BLK], f32, tag="ps")
                for q in range(BLK // MM_N):
                    for cb in range(2):
                        nc.tensor.matmul(
                            ps5[64 * cb : 64 * cb + LAST_IN, q * MM_N : (q + 1) * MM_N],
                            w5_t[64 * cb : 64 * cb + LAST_IN, :],
                            in5[64 * cb : 64 * cb + LAST_IN, 1 + q * MM_N : 1 + (q + 1) * MM_N],
                            start=True,
                            stop=True,
                        )
                nc.vector.scalar_tensor_tensor(
                    out=o5[:, :],
                    in0=lr5[:, :],
                    scalar=coef,
                    in1=ps5[:, :],
                    op0=mult,
                    op1=add,
                )
                for cb in range(2):
                    nc.scalar.dma_start(
                        out=out[LAST_R0 : LAST_R0 + LAST_OUT, BLK * cb : BLK * (cb + 1)],
                        in_=o5[64 * cb + 1 : 64 * cb + 1 + LAST_OUT, :],
                    )

            full_tile(0)
            full_tile(1)
            full_tile(2)
            full_tile(3)
            last_tile()

    nc.compile()
    return nc


def _get_nc():
    if "nc" not in _CACHE:
        _CACHE["nc"] = _build_nc()
    return _CACHE["nc"]


def _tridiag(n, a, c):
    w = np.zeros((n, n), dtype=np.float32)
    i = np.arange(n)
    w[i, i] = c
    w[i[:-1], i[1:]] = a  # k = m-1 (up neighbor)
    w[i[1:], i[:-1]] = a  # k = m+1 (down neighbor)
    return w


def _weight_inputs(a, c):
    w_main = np.empty((128, 129), dtype=np.float32)
    w_main[:, 0:128] = _tridiag(128, a, c)
    w_main[:, 128] = a  # per-partition STT coefficient
    w_aux = np.zeros((128, LAST_IN), dtype=np.float32)
    for cb in range(2):
        w_aux[64 * cb : 64 * cb + LAST_IN, :] = _tridiag(LAST_IN, a, c)
    return {"w_main": w_main, "w_aux": w_aux}


def kernel(u_main, u_coupled=None, D_eff=None, dirichlet_val=None, stencil=None,
           t=None, **_ignored):
    u = np.asarray(u_main, dtype=np.float32)
    assert u.shape == (NX, NY), u.shape
    D = float(np.asarray(D_eff).reshape(-1)[0])
    st = np.asarray(stencil).reshape(-1)
    s0, s1 = float(st[0]), float(st[1])
    dv = np.asarray(dirichlet_val, dtype=np.float32).reshape(-1)
    a = np.float32(D * s0)
    c = np.float32(4.0 * D * s1)

    S = np.empty((NX + 2, NY + 2), dtype=np.float32)
    S[1:-1, 1:-1] = u
    S[0, :] = dv[0]       # x- boundary (row 0 up-neighbor)
    S[-1, :] = dv[1]      # x+ boundary
    S[1:-1, 0] = dv[2]    # y- boundary
    S[1:-1, -1] = dv[3]   # y+ boundary

    in_maps = [
        {
            "s_in": np.ascontiguousarray(S[ROWS_PER_CORE * k : ROWS_PER_CORE * k + SLAB_ROWS]),
            **_weight_inputs(a, c),
        }
        for k in range(N_CORES)
    ]

    from concourse.bass_utils import run_bass_kernel_spmd

    res = run_bass_kernel_spmd(_get_nc(), in_maps, core_ids=list(range(N_CORES)))
    return np.concatenate([r["out"] for r in res.results], axis=0)
